# revision 1
# baseline (speedup 1.0000x reference)
"""Trainium2 Bass kernel for the DeepFermi deconvolution GD problem.

Reference computation (see problem statement): 10 fixed-step GD iterations on
a per-pixel objective

    F(eta) = ||ctc_dc - conv(aif_os, fermi_ir(eta))[::8]/8||^2 / C_dc
             + softplus(lambda) * ||(eta - eta_nn)||^2_Cnn + ||relu(-eta)||^2

The time-axis convolution with the (input-derived, iteration-independent) AIF
is a fixed 64x512 matrix M2; its transpose-products give all needed
reductions:

    s1    = sigmoid(k*(t0 - tsh))                 per pixel, [512]
    q     = M2 @ s1;   ctc_est = A*q
    r2    = (2/C_dc) * (A*q - ctc_dc)             [64]
    gA    = sum_j r2*q
    sd    = s1*(1-s1)
    U     = sum_j r2*(M2 @ sd);  V = sum_j r2*(M2V @ sd)   (M2V = M2*tsh)
    gk    = A*(t0*U - V);  gt0 = A*k*U

All pixels are independent; H(=128 rows) is sharded over the 8 cores, 16 rows
(2048 pixels) per core.  On-chip layout: pixels on partitions (one SBUF tile =
128 pixels x 512 time), sigmoid via one ScalarE activation with per-partition
scale/bias, PE transposes to feed the fixed-matrix matmuls, fused DVE
multiply-reduce ops for the dot products.
"""

import numpy as np

OSAMP = 8
MAX_ITER = 10
NEG_SHIFT = 2 * OSAMP
OTP = 5
C_SHARP = 500.0
LR = 0.1
T = 64
TOS = OSAMP * T  # 512
H = 128
W = 128
N_CORES = 8
ROWS_PER_CORE = H // N_CORES  # 16
TILES = ROWS_PER_CORE  # one 128-pixel tile per local H row
P = 128  # partitions


# ---------------------------------------------------------------------------
# host-side math (iteration independent; exact mirror of the reference's
# jax.image.resize 'linear' semantics)
# ---------------------------------------------------------------------------

def _resize_mat(in_size, out_size):
    """Column-stochastic linear-interp matrix [in, out] matching
    jax.image.resize(method='linear') for upsampling (antialias inactive)."""
    scale = out_size / in_size
    sample_f = (np.arange(out_size) + 0.5) / scale - 0.5
    x = np.abs(sample_f[None, :] - np.arange(in_size)[:, None])
    w = np.maximum(0.0, 1.0 - x)
    tot = w.sum(0, keepdims=True)
    w = np.where(np.abs(tot) > 1e-4, w / tot, 0.0)
    return w  # float64


def _sigmoid(x):
    return 1.0 / (1.0 + np.exp(-x))


def _preprocess(ctc, aif, time, eta_nn, lambda_reg):
    f64 = np.float64
    R = _resize_mat(T, TOS)
    aif0 = (aif.astype(f64) - aif.astype(f64)[..., :OTP].mean(-1, keepdims=True))
    ctc0 = (ctc.astype(f64) - ctc.astype(f64)[..., :OTP].mean(-1, keepdims=True))
    aif_os = (aif0 @ R)[0, 0, 0]                    # [512]
    t_os = time.astype(f64) @ R                     # [512]
    ctc_dc = (ctc0 @ R[:, ::OSAMP])[0]              # [H,W,64]
    C_dc = float((ctc_dc.astype(np.float32) ** 2).sum(dtype=np.float64))
    tsh = t_os - t_os[NEG_SHIFT]
    # fp32-faithful sharp step (saturates exactly like the fp32 reference)
    s2 = _sigmoid((C_SHARP * tsh).astype(np.float32).astype(f64))
    idx = NEG_SHIFT + 8 * np.arange(T)[:, None] - np.arange(TOS)[None, :]
    valid = (idx >= 0) & (idx <= TOS - 1)
    M = np.where(valid, aif_os[np.clip(idx, 0, TOS - 1)], 0.0) / OSAMP  # [64,512]
    M2 = M * s2[None, :]
    M2V = M2 * tsh[None, :]
    C_nn = (eta_nn.astype(f64) ** 2).sum(axis=(0, 2, 3))  # [3]
    sp_lam = np.logaddexp(0.0, float(lambda_reg.reshape(-1)[0]))
    creg = 2.0 * sp_lam / C_nn                      # [3]
    return M2, M2V, tsh, ctc_dc, C_dc, creg


# ---------------------------------------------------------------------------
# bass module (input-value independent; all data arrives via DRAM tensors)
# ---------------------------------------------------------------------------

_NC_CACHE = {}


def _build_nc():
    if "nc" in _NC_CACHE:
        return _NC_CACHE["nc"]

    import concourse.mybir as mybir
    import concourse.tile as tile
    from concourse import bacc

    dt = mybir.dt.float32
    bf = mybir.dt.bfloat16
    Alu = mybir.AluOpType
    Act = mybir.ActivationFunctionType

    nc = bacc.Bacc("TRN2", target_bir_lowering=False, debug=False)

    # shared constants (identical on every core)
    d_argw = nc.declare_dram_parameter("argw", [2 * TILES, 4 * TILES * P], bf,
                                       isOutput=False)
    d_ident = nc.declare_dram_parameter("ident", [P, P], bf, isOutput=False)
    d_m2t = nc.declare_dram_parameter("m2t", [P, 4 * T], bf, isOutput=False)
    d_muv = nc.declare_dram_parameter("muv", [P, 4 * 2 * T], bf, isOutput=False)
    # per-core data
    d_nctc = nc.declare_dram_parameter("negctc2", [P, TILES * T], dt, isOutput=False)
    d_eta0 = nc.declare_dram_parameter("eta0", [P, 3 * TILES], dt, isOutput=False)
    d_cpl48 = nc.declare_dram_parameter("cpl48", [P, 3 * TILES], dt, isOutput=False)
    d_s48 = nc.declare_dram_parameter("s48", [P, 3 * TILES], dt, isOutput=False)
    d_consts = nc.declare_dram_parameter("consts", [P, TILES], dt, isOutput=False)
    d_out = nc.declare_dram_parameter("out", [P, 3 * TILES], dt, isOutput=True)

    with tile.TileContext(nc) as tc:
        with (
            tc.tile_pool(name="const", bufs=1) as cpool,
            tc.tile_pool(name="state", bufs=2) as spool,
            tc.tile_pool(name="work", bufs=5) as wpool,
            tc.tile_pool(name="tpose", bufs=2) as tpool,
            tc.tile_pool(name="small", bufs=2) as mpool,
            tc.tile_pool(name="ps_t", bufs=3, space="PSUM") as ps_t,
            tc.tile_pool(name="ps_q", bufs=4, space="PSUM") as ps_q,
            tc.tile_pool(name="ps_k", bufs=1, space="PSUM") as ps_k,
        ):
            # ---- load constants ----
            argw = cpool.tile([2 * TILES, 4 * TILES * P], bf, tag="argw")
            nc.gpsimd.dma_start(argw[:], d_argw[:])
            ident = cpool.tile([P, P], bf, tag="ident")
            nc.gpsimd.dma_start(ident[:], d_ident[:])
            m2t = cpool.tile([P, 4 * T], bf, tag="m2t")
            nc.gpsimd.dma_start(m2t[:], d_m2t[:])
            muv = cpool.tile([P, 8 * T], bf, tag="muv")
            nc.gpsimd.dma_start(muv[:], d_muv[:])
            nctc = cpool.tile([P, TILES * T], dt, tag="nctc")
            nc.gpsimd.dma_start(nctc[:], d_nctc[:])
            cpl48 = cpool.tile([P, 3 * TILES], dt, tag="cpl48")
            nc.gpsimd.dma_start(cpl48[:], d_cpl48[:])
            s48 = cpool.tile([P, 3 * TILES], dt, tag="s48")
            nc.gpsimd.dma_start(s48[:], d_s48[:])
            consts = cpool.tile([P, TILES], dt, tag="consts")
            nc.gpsimd.dma_start(consts[:], d_consts[:])
            eta_in = cpool.tile([P, 3 * TILES], dt, tag="eta_in")
            nc.gpsimd.dma_start(eta_in[:], d_eta0[:])

            toc16 = consts[:, 0:TILES]

            # ---- initial eta state (A|k|t0 packed) + derived tiles ----
            eta48 = spool.tile([P, 3 * TILES], dt, tag="eta48")
            nc.vector.tensor_copy(eta48[:], eta_in[:])

            def make_derived(e48):
                eA = e48[:, 0:TILES]
                eK = e48[:, TILES:2 * TILES]
                eT = e48[:, 2 * TILES:3 * TILES]
                # kn[:, 2t] = (k*t0)_t, kn[:, 2t+1] = (-k)_t  (bf16), then
                # transpose so tile t's arg-matmul rhs is knT[2t:2t+2, :]
                kn = spool.tile([P, 2 * TILES], bf, tag="kn")
                nc.vector.tensor_tensor(kn[:, 0:2 * TILES:2], eK, eT,
                                        Alu.mult)
                nc.vector.tensor_scalar_mul(kn[:, 1:2 * TILES:2], eK, -1.0)
                knt_ps = ps_k.tile([2 * TILES, P], bf, tag="kntp")
                nc.tensor.transpose(knt_ps[:], kn[:], ident[:])
                knT = spool.tile([2 * TILES, P], bf, tag="knT")
                nc.scalar.copy(knT[:], knt_ps[:])
                a2c = spool.tile([P, TILES], dt, tag="a2c")
                nc.vector.tensor_tensor(a2c[:], eA[:], toc16, Alu.mult)
                return knT, a2c

            knT, a2c = make_derived(eta48)

            for it in range(MAX_ITER):
                G48 = mpool.tile([P, 3 * TILES], dt, tag="G48")
                accGA = G48[:, 0:TILES]
                accU = mpool.tile([P, TILES], dt, tag="accU")
                accV = mpool.tile([P, TILES], dt, tag="accV")

                for t in range(TILES):
                    # argT[v,p] = kt0_p - k_p*tsh_v via rank-2 matmul:
                    # lhsT = [ones; tsh] chunk, rhs = knT[2t:2t+2, :]
                    argp = ps_t.tile([P, TOS], dt, tag="argp")
                    for c in range(4):
                        blk = 4 * t + c
                        nc.tensor.matmul(
                            argp[:, c * P:(c + 1) * P],
                            argw[:, blk * P:(blk + 1) * P],
                            knT[:],
                            start=True, stop=True,
                        )
                    # s1T = sigmoid(argT)  (PSUM -> SBUF, bf16 out)
                    s1T = wpool.tile([P, TOS], bf, tag="s1T")
                    nc.scalar.activation(s1T[:], argp[:], Act.Sigmoid)
                    # sdT = s1T*(1-s1T)
                    sdT = wpool.tile([P, TOS], bf, tag="sdT")
                    sdacc = wpool.tile([P, 1], dt, tag="sdacc")
                    nc.vector.affine_mul_reduce(
                        sdT[:], sdacc[:], s1T[:], s1T[:], -1.0, 1.0,
                    )

                    # q = M2 @ s1 -> [128p, 64]; qd|qdv = (M2|M2V) @ sd -> [128p, 128]
                    qq = ps_q.tile([P, 3 * T], dt, tag="qq")
                    for c in range(4):
                        nc.tensor.matmul(
                            qq[:, 0:T], s1T[:, c * P:(c + 1) * P],
                            m2t[:, c * T:(c + 1) * T],
                            start=(c == 0), stop=(c == 3),
                        )
                    for c in range(4):
                        nc.tensor.matmul(
                            qq[:, T: 3 * T], sdT[:, c * P:(c + 1) * P],
                            muv[:, c * 2 * T:(c + 1) * 2 * T],
                            start=(c == 0), stop=(c == 3),
                        )
                    # single PSUM->SBUF copy (bf16) for all of q|qd|qdv
                    qqs = wpool.tile([P, 3 * T], bf, tag="qqs")
                    nc.scalar.copy(qqs[:], qq[:])
                    q_ap = qqs[:, 0:T]
                    qd_ap = qqs[:, T: 2 * T]
                    qdv_ap = qqs[:, 2 * T: 3 * T]

                    # r2 = (2A/C_dc)*q - (2/C_dc)*ctc_dc
                    r2 = wpool.tile([P, T], dt, tag="r2")
                    nc.vector.affine_then_add(
                        r2[:], q_ap, nctc[:, t * T:(t + 1) * T],
                        a2c[:, t:t + 1], 0.0,
                    )
                    # dots: accGA[:,t] = sum r2*q ; accU ; accV   (seed 0;
                    # the cpa prior-term is added during the combine phase)
                    dsc = wpool.tile([P, 3 * T], dt, tag="dsc")
                    nc.vector.affine_mul_reduce(
                        dsc[:, 0:T], accGA[:, t:t + 1], q_ap, r2[:], 1.0, 0.0)
                    nc.vector.affine_mul_reduce(
                        dsc[:, T: 2 * T], accU[:, t:t + 1], qd_ap, r2[:], 1.0, 0.0)
                    nc.vector.affine_mul_reduce(
                        dsc[:, 2 * T: 3 * T], accV[:, t:t + 1], qdv_ap, r2[:], 1.0, 0.0)

                # ---- combine: eta <- eta - LR*grad, batched [128,48] ----
                # products chain (GpSimd, idle engine): G48 cols 16:48
                eA = eta48[:, 0:TILES]
                eK = eta48[:, TILES:2 * TILES]
                eT = eta48[:, 2 * TILES:3 * TILES]
                p1 = mpool.tile([P, TILES], dt, tag="p1")
                nc.gpsimd.tensor_tensor(p1[:], eA, accU[:], Alu.mult)
                p2 = mpool.tile([P, TILES], dt, tag="p2")
                nc.gpsimd.tensor_tensor(p2[:], eA, accV[:], Alu.mult)
                wk = mpool.tile([P, TILES], dt, tag="wk")
                nc.gpsimd.tensor_tensor(wk[:], eT, p1[:], Alu.mult)
                nc.gpsimd.tensor_tensor(G48[:, TILES:2 * TILES], wk[:], p2[:],
                                        Alu.subtract)
                nc.gpsimd.tensor_tensor(G48[:, 2 * TILES:3 * TILES], p1[:], eK,
                                        Alu.mult)
                # DVE: m48 = -2LR*min(eta,0); eta' = eta*s48 - LR*G48 + m48 + cpl48
                m48 = mpool.tile([P, 3 * TILES], dt, tag="m48")
                nc.vector.tensor_scalar(m48[:], eta48[:], 0.0, -2.0 * LR,
                                        Alu.min, Alu.mult)
                t48 = mpool.tile([P, 3 * TILES], dt, tag="t48")
                nc.vector.affine_then_add(t48[:], G48[:], m48[:], -LR, 0.0)
                t48b = mpool.tile([P, 3 * TILES], dt, tag="t48b")
                nc.vector.tensor_tensor(t48b[:], t48[:], cpl48[:], Alu.add)
                up48 = mpool.tile([P, 3 * TILES], dt, tag="up48")
                nc.vector.tensor_tensor(up48[:], eta48[:], s48[:], Alu.mult)
                eta48n = spool.tile([P, 3 * TILES], dt, tag="eta48")
                nc.vector.tensor_tensor(eta48n[:], up48[:], t48b[:], Alu.add)

                eta48 = eta48n
                if it < MAX_ITER - 1:
                    knT, a2c = make_derived(eta48)

            nc.gpsimd.dma_start(d_out[:], eta48[:])

    nc.finalize()
    _NC_CACHE["nc"] = nc
    return nc


# ---------------------------------------------------------------------------
# public entry point
# ---------------------------------------------------------------------------

def _make_in_maps(ctc, aif, time, eta_nn, lambda_reg):
    f32 = np.float32
    M2, M2V, tsh, ctc_dc, C_dc, creg = _preprocess(ctc, aif, time, eta_nn, lambda_reg)

    toc = 2.0 / C_dc
    sA, sK, sT0 = (1.0 - LR * creg).astype(np.float64)

    import ml_dtypes
    bf16 = ml_dtypes.bfloat16
    # argw[r, 128*(4t+c)+vv] = 1 if r==2t else tsh[128c+vv] if r==2t+1 else 0
    argw = np.zeros((2 * TILES, 4 * TILES * P), bf16)
    tshf = tsh.astype(np.float32)
    for t_ in range(TILES):
        for c_ in range(4):
            blk = 4 * t_ + c_
            argw[2 * t_, blk * P:(blk + 1) * P] = 1.0
            argw[2 * t_ + 1, blk * P:(blk + 1) * P] = tshf[c_ * P:(c_ + 1) * P]
    ident = np.eye(P, dtype=bf16)
    # m2t[vv, 64c+j] = M2[j, 128c+vv];  muv[vv, 128c+j'] = (M2|M2V)[j', 128c+vv]
    m2t = np.zeros((P, 4 * T), bf16)
    muv = np.zeros((P, 8 * T), bf16)
    for c in range(4):
        blk = M2[:, c * P:(c + 1) * P]       # [64,128]
        blkv = M2V[:, c * P:(c + 1) * P]
        m2t[:, c * T:(c + 1) * T] = blk.T
        muv[:, c * 2 * T: c * 2 * T + T] = blk.T
        muv[:, c * 2 * T + T: (c + 1) * 2 * T] = blkv.T

    consts = np.full((P, TILES), toc, f32)
    s48 = np.zeros((P, 3 * TILES), f32)
    s48[:, 0:TILES] = sA
    s48[:, TILES:2 * TILES] = sK
    s48[:, 2 * TILES:] = sT0

    in_maps = []
    for m in range(N_CORES):
        rows = slice(m * ROWS_PER_CORE, (m + 1) * ROWS_PER_CORE)
        # ctc_dc[h, w, j]: tile t = local row, partition p = w
        cd = ctc_dc[rows]                     # [16, 128, 64]
        negctc2 = np.ascontiguousarray(
            (-toc * cd).transpose(1, 0, 2).reshape(P, TILES * T)).astype(f32)
        pr = eta_nn[0, :, rows, :].astype(np.float64)   # [3, 16, 128] (c, t, p)
        eta0 = np.ascontiguousarray(
            pr.transpose(2, 0, 1).reshape(P, 3 * TILES)).astype(f32)
        cpl48 = np.zeros((P, 3 * TILES), f32)
        for c in range(3):
            cpl48[:, c * TILES:(c + 1) * TILES] = (LR * creg[c] * pr[c]).T
        in_maps.append({
            "argw": argw, "ident": ident, "m2t": m2t, "muv": muv,
            "negctc2": negctc2, "eta0": eta0, "cpl48": cpl48, "s48": s48,
            "consts": consts,
        })
    return in_maps


def kernel(ctc, aif, time, seg, eta_nn, lambda_reg):
    from concourse.bass_utils import run_bass_kernel_spmd

    ctc = np.asarray(ctc)
    aif = np.asarray(aif)
    time = np.asarray(time)
    eta_nn = np.asarray(eta_nn)
    lambda_reg = np.asarray(lambda_reg)

    in_maps = _make_in_maps(ctc, aif, time, eta_nn, lambda_reg)
    nc = _build_nc()
    res = run_bass_kernel_spmd(nc, in_maps, list(range(N_CORES)))

    out = np.zeros((1, 3, H, W), np.float32)
    for m in range(N_CORES):
        rows = slice(m * ROWS_PER_CORE, (m + 1) * ROWS_PER_CORE)
        arr = res.results[m]["out"]                  # [128, 48]
        out[0, :, rows, :] = arr.reshape(P, 3, TILES).transpose(1, 2, 0)
    return out



# revision 5
# speedup vs baseline: 2.1430x; 2.1430x over previous
"""Trainium2 Bass kernel for the DeepFermi deconvolution GD problem.

Node-collapsed formulation: the per-pixel fermi sigmoid s1(tsh) is smooth on
the >=1 time-unit scale (k <= ~1), so the 512-point oversampled time grid is
replaced by Nc=16 interpolation nodes tshc; the piecewise-linear interp
matrix L is folded host-side into the convolution matrices:

    ML = M2 @ L, MVL = M2V @ L
    G  = ML^T ML,  Gv^T = ML^T MVL          (64x64 -> Nc x Nc Gram matrices)
    nw  = -(2/C_dc) ctc_dc @ ML             (per-pixel constants)
    nwv = -(2/C_dc) ctc_dc @ MVL

Per GD iteration, per pixel (s1c/sdc on the Nc nodes):
    w  = a2c*(G @ s1c)  + nw                (a2c = 2A/C_dc)
    wv = a2c*(Gv @ s1c) + nwv
    gA = w.s1c ; U = w.sdc ; V = wv.sdc
    gk = A(t0 U - V) ; gt0 = A k U          (+ prior & positivity terms)

Mapping: H sharded over 8 cores (16 rows each); tile = one H row = 128
pixels on partitions.  Tiles processed in groups of 4 via block-diagonal
matmuls: one PE matmul each for argT ([64,128] = 4x16 nodes x pixels),
arg2 ([128, 64] pixels x nodes), and y|yv ([128,128] via block-diag Gram),
ScalarE sigmoids in both layouts + per-tile a2c scaling, DVE for sdc,
w-add, dot products (segmented tensor_reduce), and the GD update.
"""

import numpy as np

OSAMP = 8
MAX_ITER = 10
NEG_SHIFT = 2 * OSAMP
OTP = 5
C_SHARP = 500.0
LR = 0.1
T = 64
TOS = OSAMP * T  # 512
H = 128
W = 128
N_CORES = 8
ROWS_PER_CORE = H // N_CORES  # 16
TILES = ROWS_PER_CORE  # 16 tiles of 128 pixels per core
P = 128
NC = 16        # interpolation nodes
GRP = 4        # tiles per group
NGRP = TILES // GRP  # 4 groups


# ---------------------------------------------------------------------------
# host-side math (iteration independent)
# ---------------------------------------------------------------------------

def _resize_mat(in_size, out_size):
    scale = out_size / in_size
    sample_f = (np.arange(out_size) + 0.5) / scale - 0.5
    x = np.abs(sample_f[None, :] - np.arange(in_size)[:, None])
    w = np.maximum(0.0, 1.0 - x)
    tot = w.sum(0, keepdims=True)
    w = np.where(np.abs(tot) > 1e-4, w / tot, 0.0)
    return w  # float64


def _sigmoid(x):
    with np.errstate(over="ignore"):
        return 1.0 / (1.0 + np.exp(-x))


def _interp_mat(tsh, tshc):
    """L [512, Nc]: piecewise-linear interp from nodes tshc to grid tsh,
    clamped extrapolation."""
    Nc = len(tshc)
    Lm = np.zeros((len(tsh), Nc))
    j = np.searchsorted(tshc, tsh)
    for v, (x, jj) in enumerate(zip(tsh, j)):
        if jj <= 0:
            Lm[v, 0] = 1.0
        elif jj >= Nc:
            Lm[v, Nc - 1] = 1.0
        else:
            x0, x1 = tshc[jj - 1], tshc[jj]
            a = (x - x0) / (x1 - x0)
            Lm[v, jj - 1] = 1.0 - a
            Lm[v, jj] = a
    return Lm


def _preprocess(ctc, aif, time, eta_nn, lambda_reg):
    f64 = np.float64
    R = _resize_mat(T, TOS)
    aif0 = (aif.astype(f64) - aif.astype(f64)[..., :OTP].mean(-1, keepdims=True))
    ctc0 = (ctc.astype(f64) - ctc.astype(f64)[..., :OTP].mean(-1, keepdims=True))
    aif_os = (aif0 @ R)[0, 0, 0]                    # [512]
    t_os = time.astype(f64) @ R                     # [512]
    ctc_dc = (ctc0 @ R[:, ::OSAMP])[0]              # [H,W,64]
    C_dc = float((ctc_dc.astype(np.float32) ** 2).sum(dtype=np.float64))
    tsh = t_os - t_os[NEG_SHIFT]
    s2 = _sigmoid((C_SHARP * tsh).astype(np.float32).astype(f64))
    idx = NEG_SHIFT + 8 * np.arange(T)[:, None] - np.arange(TOS)[None, :]
    valid = (idx >= 0) & (idx <= TOS - 1)
    M = np.where(valid, aif_os[np.clip(idx, 0, TOS - 1)], 0.0) / OSAMP  # [64,512]
    M2 = M * s2[None, :]
    M2V = M2 * tsh[None, :]

    import ml_dtypes
    nidx = np.round(np.linspace(0, TOS - 1, NC)).astype(int)
    # bf16-representable node values so device arg matches host L exactly
    tshc = tsh[nidx].astype(ml_dtypes.bfloat16).astype(f64)
    Lm = _interp_mat(tsh, tshc)
    ML = M2 @ Lm                                    # [64, NC]
    MVL = M2V @ Lm
    G = ML.T @ ML                                   # [NC, NC] symmetric
    GvT = ML.T @ MVL                                # = Gv^T, y_v = s1c @ GvT
    nw = -(2.0 / C_dc) * np.einsum('hwj,jc->hwc', ctc_dc, ML)    # [H,W,NC]
    nwv = -(2.0 / C_dc) * np.einsum('hwj,jc->hwc', ctc_dc, MVL)

    C_nn = (eta_nn.astype(f64) ** 2).sum(axis=(0, 2, 3))  # [3]
    sp_lam = np.logaddexp(0.0, float(lambda_reg.reshape(-1)[0]))
    creg = 2.0 * sp_lam / C_nn                      # [3]
    return tshc, G, GvT, nw, nwv, C_dc, creg


# ---------------------------------------------------------------------------
# bass module
# ---------------------------------------------------------------------------

_NC_CACHE = {}


def _build_nc():
    if "nc" in _NC_CACHE:
        return _NC_CACHE["nc"]

    import concourse.mybir as mybir
    import concourse.tile as tile
    from concourse import bacc

    dt = mybir.dt.float32
    bf = mybir.dt.bfloat16
    Alu = mybir.AluOpType
    Act = mybir.ActivationFunctionType

    nc = bacc.Bacc("TRN2", target_bir_lowering=False, debug=False)

    # shared constants
    d_argwbd = nc.declare_dram_parameter("argwbd", [2 * TILES, NGRP * GRP * NC],
                                         bf, isOutput=False)
    d_ggvd = nc.declare_dram_parameter("ggvd", [GRP * NC, GRP * 2 * NC], bf,
                                       isOutput=False)
    d_ident = nc.declare_dram_parameter("ident", [P, P], bf, isOutput=False)
    # per-core data
    d_nwfull = nc.declare_dram_parameter("nwfull", [P, TILES * 2 * NC], bf,
                                         isOutput=False)
    d_eta0 = nc.declare_dram_parameter("eta0", [P, 3 * TILES], dt, isOutput=False)
    d_cpl48 = nc.declare_dram_parameter("cpl48", [P, 3 * TILES], dt, isOutput=False)
    d_s48 = nc.declare_dram_parameter("s48", [P, 3 * TILES], dt, isOutput=False)
    d_toc = nc.declare_dram_parameter("toc", [P, TILES], dt, isOutput=False)
    d_out = nc.declare_dram_parameter("out", [P, 3 * TILES], dt, isOutput=True)

    with tile.TileContext(nc) as tc:
        with (
            tc.tile_pool(name="const", bufs=1) as cpool,
            tc.tile_pool(name="state", bufs=2) as spool,
            tc.tile_pool(name="work", bufs=3) as wpool,
            tc.tile_pool(name="small", bufs=2) as mpool,
            tc.tile_pool(name="ps_a", bufs=2, space="PSUM") as ps_a,
            tc.tile_pool(name="ps_b", bufs=2, space="PSUM") as ps_b,
            tc.tile_pool(name="ps_y", bufs=2, space="PSUM") as ps_y,
            tc.tile_pool(name="ps_k", bufs=1, space="PSUM") as ps_k,
        ):
            # ---- load constants ----
            argwbd = cpool.tile([2 * TILES, NGRP * GRP * NC], bf, tag="argwbd")
            nc.gpsimd.dma_start(argwbd[:], d_argwbd[:])
            ggvd = cpool.tile([GRP * NC, GRP * 2 * NC], bf, tag="ggvd")
            nc.gpsimd.dma_start(ggvd[:], d_ggvd[:])
            ident = cpool.tile([P, P], bf, tag="ident")
            nc.gpsimd.dma_start(ident[:], d_ident[:])
            nwfull = cpool.tile([P, TILES * 2 * NC], bf, tag="nwfull")
            nc.gpsimd.dma_start(nwfull[:], d_nwfull[:])
            cpl48 = cpool.tile([P, 3 * TILES], dt, tag="cpl48")
            nc.gpsimd.dma_start(cpl48[:], d_cpl48[:])
            s48 = cpool.tile([P, 3 * TILES], dt, tag="s48")
            nc.gpsimd.dma_start(s48[:], d_s48[:])
            toc = cpool.tile([P, TILES], dt, tag="toc")
            nc.gpsimd.dma_start(toc[:], d_toc[:])
            eta_in = cpool.tile([P, 3 * TILES], dt, tag="eta_in")
            nc.gpsimd.dma_start(eta_in[:], d_eta0[:])

            eta48 = spool.tile([P, 3 * TILES], dt, tag="eta48")
            nc.vector.tensor_copy(eta48[:], eta_in[:])

            def make_derived(e48):
                eA = e48[:, 0:TILES]
                eK = e48[:, TILES:2 * TILES]
                eT = e48[:, 2 * TILES:3 * TILES]
                # kn[:, 2t] = (k*t0)_t, kn[:, 2t+1] = (-k)_t (bf16)
                kn = spool.tile([P, 2 * TILES], bf, tag="kn")
                nc.vector.tensor_tensor(kn[:, 0:2 * TILES:2], eK, eT, Alu.mult)
                nc.vector.tensor_scalar_mul(kn[:, 1:2 * TILES:2], eK, -1.0)
                knt_ps = ps_k.tile([2 * TILES, P], bf, tag="kntp")
                nc.tensor.transpose(knt_ps[:], kn[:], ident[:])
                knT = spool.tile([2 * TILES, P], bf, tag="knT")
                nc.scalar.copy(knT[:], knt_ps[:])
                # a2c[:, t] = (2/C_dc) * A_t  (per-pixel scale, fp32)
                a2c = spool.tile([P, TILES], dt, tag="a2c")
                nc.vector.tensor_tensor(a2c[:], eA[:], toc[:], Alu.mult)
                return knT, a2c

            knT, a2c = make_derived(eta48)

            for it in range(MAX_ITER):
                UVm = mpool.tile([P, 2 * TILES], dt, tag="UVm")
                G48 = mpool.tile([P, 3 * TILES], dt, tag="G48")

                for g in range(NGRP):
                    awg = argwbd[:, g * GRP * NC:(g + 1) * GRP * NC]
                    # argT[16*tau + c, pix] = kt0 - k*tshc[c] for tile 4g+tau
                    argT = ps_a.tile([GRP * NC, P], dt, tag="argT")
                    nc.tensor.matmul(argT[:], awg, knT[:], start=True, stop=True)
                    # arg2[pix, 16*tau + c] = same, pixel-major
                    arg2 = ps_b.tile([P, GRP * NC], dt, tag="arg2")
                    nc.tensor.matmul(arg2[:], knT[:], awg, start=True, stop=True)
                    # sigmoids in both layouts
                    s1cT = wpool.tile([GRP * NC, P], bf, tag="s1cT")
                    nc.scalar.activation(s1cT[:], argT[:], Act.Sigmoid)
                    s1c = wpool.tile([P, GRP * NC], bf, tag="s1c")
                    nc.scalar.activation(s1c[:], arg2[:], Act.Sigmoid)
                    # y|yv block-diag matmul: [pix, 32*tau + (c'|c'v)]
                    yps = ps_y.tile([P, GRP * 2 * NC], dt, tag="yps")
                    nc.tensor.matmul(yps[:], s1cT[:], ggvd[:], start=True,
                                     stop=True)
                    # per-tile a2c scaling (PSUM -> SBUF, bf16)
                    ya = wpool.tile([P, GRP * 2 * NC], bf, tag="ya")
                    for tau in range(GRP):
                        t_ = g * GRP + tau
                        nc.scalar.mul(ya[:, tau * 2 * NC:(tau + 1) * 2 * NC],
                                      yps[:, tau * 2 * NC:(tau + 1) * 2 * NC],
                                      a2c[:, t_:t_ + 1])
                    # sdc = s1c*(1-s1c)
                    sdc = wpool.tile([P, GRP * NC], bf, tag="sdc")
                    sdacc = wpool.tile([P, 1], dt, tag="sdacc")
                    nc.vector.affine_mul_reduce(sdc[:], sdacc[:], s1c[:],
                                                s1c[:], -1.0, 1.0)
                    # w|wv = ya + nw|nwv
                    wg = wpool.tile([P, GRP * 2 * NC], bf, tag="wg")
                    nc.vector.tensor_tensor(
                        wg[:], ya[:],
                        nwfull[:, g * GRP * 2 * NC:(g + 1) * GRP * 2 * NC],
                        Alu.add)
                    # products + segmented reduces
                    w4 = wg[:].rearrange("p (t u c) -> p t u c", t=GRP, u=2,
                                         c=NC)
                    sdc3 = sdc[:].rearrange("p (t c) -> p t c", t=GRP)
                    sdc4 = sdc3.unsqueeze(2).broadcast_to([P, GRP, 2, NC])
                    pA = wpool.tile([P, GRP * 2 * NC], bf, tag="pA")
                    pA4 = pA[:].rearrange("p (t u c) -> p t u c", t=GRP, u=2,
                                          c=NC)
                    nc.vector.tensor_tensor(pA4, w4, sdc4, Alu.mult)
                    pB = wpool.tile([P, GRP * NC], bf, tag="pB")
                    pB3 = pB[:].rearrange("p (t c) -> p t c", t=GRP)
                    s1c3 = s1c[:].rearrange("p (t c) -> p t c", t=GRP)
                    nc.vector.tensor_tensor(pB3, w4[:, :, 0, :], s1c3, Alu.mult)
                    uv = UVm[:, g * 2 * GRP:(g + 1) * 2 * GRP]
                    nc.vector.reduce_sum(
                        uv.rearrange("p (t u) -> p t u", t=GRP), pA4,
                        axis=mybir.AxisListType.X)
                    nc.vector.reduce_sum(
                        G48[:, g * GRP:(g + 1) * GRP], pB3,
                        axis=mybir.AxisListType.X)

                # ---- combine: eta <- eta - LR*grad, batched [128,48] ----
                eA = eta48[:, 0:TILES]
                eK = eta48[:, TILES:2 * TILES]
                eT = eta48[:, 2 * TILES:3 * TILES]
                Um = UVm[:, 0:2 * TILES:2]
                Vm = UVm[:, 1:2 * TILES:2]
                p1 = mpool.tile([P, TILES], dt, tag="p1")
                nc.vector.tensor_tensor(p1[:], eA, Um, Alu.mult)
                p2 = mpool.tile([P, TILES], dt, tag="p2")
                nc.vector.tensor_tensor(p2[:], eA, Vm, Alu.mult)
                wk = mpool.tile([P, TILES], dt, tag="wk")
                nc.vector.tensor_tensor(wk[:], eT, p1[:], Alu.mult)
                nc.vector.tensor_tensor(G48[:, TILES:2 * TILES], wk[:], p2[:],
                                        Alu.subtract)
                nc.vector.tensor_tensor(G48[:, 2 * TILES:3 * TILES], p1[:], eK,
                                        Alu.mult)
                # m48 = -2LR*min(eta,0); eta' = eta*s48 - LR*G48 + m48 + cpl48
                m48 = mpool.tile([P, 3 * TILES], dt, tag="m48")
                nc.vector.tensor_scalar(m48[:], eta48[:], 0.0, -2.0 * LR,
                                        Alu.min, Alu.mult)
                t48 = mpool.tile([P, 3 * TILES], dt, tag="t48")
                nc.vector.affine_then_add(t48[:], G48[:], m48[:], -LR, 0.0)
                t48b = mpool.tile([P, 3 * TILES], dt, tag="t48b")
                nc.vector.tensor_tensor(t48b[:], t48[:], cpl48[:], Alu.add)
                up48 = mpool.tile([P, 3 * TILES], dt, tag="up48")
                nc.vector.tensor_tensor(up48[:], eta48[:], s48[:], Alu.mult)
                eta48n = spool.tile([P, 3 * TILES], dt, tag="eta48")
                nc.vector.tensor_tensor(eta48n[:], up48[:], t48b[:], Alu.add)

                eta48 = eta48n
                if it < MAX_ITER - 1:
                    knT, a2c = make_derived(eta48)

            nc.gpsimd.dma_start(d_out[:], eta48[:])

    nc.finalize()
    _NC_CACHE["nc"] = nc
    return nc


# ---------------------------------------------------------------------------
# input staging
# ---------------------------------------------------------------------------

def _make_in_maps(ctc, aif, time, eta_nn, lambda_reg):
    f32 = np.float32
    import ml_dtypes
    bf16 = ml_dtypes.bfloat16

    tshc, G, GvT, nw, nwv, C_dc, creg = _preprocess(
        ctc, aif, time, eta_nn, lambda_reg)

    toc_v = 2.0 / C_dc
    sA, sK, sT0 = (1.0 - LR * creg).astype(np.float64)

    # argwbd[k, 16*(4g+tau)+c]: rows 2t -> 1.0, 2t+1 -> tshc[c] for t=4g+tau
    argwbd = np.zeros((2 * TILES, TILES * NC), bf16)
    tshcf = tshc.astype(f32)
    for t_ in range(TILES):
        argwbd[2 * t_, t_ * NC:(t_ + 1) * NC] = 1.0
        argwbd[2 * t_ + 1, t_ * NC:(t_ + 1) * NC] = tshcf
    # ggvd block-diag: rows 16*tau..+16, cols 32*tau..+32 = [G | Gv^T]
    ggvd = np.zeros((GRP * NC, GRP * 2 * NC), bf16)
    blk = np.concatenate([G, GvT], axis=1)          # [NC, 2*NC]
    for tau in range(GRP):
        ggvd[tau * NC:(tau + 1) * NC, tau * 2 * NC:(tau + 1) * 2 * NC] = blk
    ident = np.eye(P, dtype=bf16)

    toc = np.full((P, TILES), toc_v, f32)
    s48 = np.zeros((P, 3 * TILES), f32)
    s48[:, 0:TILES] = sA
    s48[:, TILES:2 * TILES] = sK
    s48[:, 2 * TILES:] = sT0

    in_maps = []
    for m in range(N_CORES):
        rows = slice(m * ROWS_PER_CORE, (m + 1) * ROWS_PER_CORE)
        # nwfull[pix, 32t + (c | 16+c)] = nw|nwv for tile t
        nwc = np.stack([nw[rows], nwv[rows]], axis=2)  # [16,128,2,NC]
        nwfull = np.ascontiguousarray(
            nwc.transpose(1, 0, 2, 3).reshape(P, TILES * 2 * NC)).astype(bf16)
        pr = eta_nn[0, :, rows, :].astype(np.float64)   # [3, 16, 128]
        eta0 = np.ascontiguousarray(
            pr.transpose(2, 0, 1).reshape(P, 3 * TILES)).astype(f32)
        cpl48 = np.zeros((P, 3 * TILES), f32)
        for c in range(3):
            cpl48[:, c * TILES:(c + 1) * TILES] = (LR * creg[c] * pr[c]).T
        in_maps.append({
            "argwbd": argwbd, "ggvd": ggvd, "ident": ident,
            "nwfull": nwfull, "eta0": eta0, "cpl48": cpl48, "s48": s48,
            "toc": toc,
        })
    return in_maps


def _emulate(in_maps):
    """Numpy replay of the device pipeline from staged arrays (debug aid)."""
    import ml_dtypes
    bf16 = ml_dtypes.bfloat16
    f32 = np.float32

    def bfq(x):
        return np.asarray(x, dtype=f32).astype(bf16).astype(f32)

    outs = []
    for mp in in_maps:
        argwbd = mp["argwbd"].astype(f32)
        ggvd = mp["ggvd"].astype(f32)
        nwfull = mp["nwfull"].astype(f32)
        eta48 = mp["eta0"].astype(f32).copy()
        cpl48 = mp["cpl48"]
        s48 = mp["s48"]
        toc = mp["toc"]
        for it in range(MAX_ITER):
            eA = eta48[:, 0:TILES]
            eK = eta48[:, TILES:2 * TILES]
            eT = eta48[:, 2 * TILES:3 * TILES]
            kn = np.zeros((P, 2 * TILES), f32)
            kn[:, 0::2] = bfq(eK * eT)
            kn[:, 1::2] = bfq(-eK)
            knT = kn.T  # [32, 128]
            a2c = eA * toc
            UVm = np.zeros((P, 2 * TILES), f32)
            GAm = np.zeros((P, TILES), f32)
            for g in range(NGRP):
                awg = argwbd[:, g * GRP * NC:(g + 1) * GRP * NC]
                argT = awg.T @ knT          # [64, 128]
                arg2 = knT.T @ awg          # [128, 64]
                s1cT = bfq(_sigmoid(argT))
                s1c = bfq(_sigmoid(arg2))
                yps = s1cT.T @ ggvd         # [128, 128]
                ya = np.zeros_like(yps)
                for tau in range(GRP):
                    t_ = g * GRP + tau
                    ya[:, tau * 2 * NC:(tau + 1) * 2 * NC] = bfq(
                        yps[:, tau * 2 * NC:(tau + 1) * 2 * NC]
                        * a2c[:, t_:t_ + 1])
                sdc = bfq(s1c * (1.0 - s1c))
                wg = bfq(ya + nwfull[:, g * GRP * 2 * NC:(g + 1) * GRP * 2 * NC])
                w4 = wg.reshape(P, GRP, 2, NC)
                pA = bfq(w4 * sdc.reshape(P, GRP, 1, NC))
                pB = bfq(w4[:, :, 0, :] * s1c.reshape(P, GRP, NC))
                UVm[:, g * 2 * GRP:(g + 1) * 2 * GRP] = pA.sum(-1).reshape(
                    P, 2 * GRP)
                GAm[:, g * GRP:(g + 1) * GRP] = pB.sum(-1)
            Um = UVm.reshape(P, TILES, 2)[:, :, 0]
            Vm = UVm.reshape(P, TILES, 2)[:, :, 1]
            G48 = np.zeros((P, 3 * TILES), f32)
            G48[:, 0:TILES] = GAm
            p1 = eA * Um
            p2 = eA * Vm
            G48[:, TILES:2 * TILES] = eT * p1 - p2
            G48[:, 2 * TILES:] = p1 * eK
            m48 = np.minimum(eta48, 0.0) * (-2.0 * LR)
            eta48 = eta48 * s48 + (G48 * (-LR) + m48) + cpl48
        outs.append(eta48)
    out = np.zeros((1, 3, H, W), f32)
    for m, arr in enumerate(outs):
        rows = slice(m * ROWS_PER_CORE, (m + 1) * ROWS_PER_CORE)
        out[0, :, rows, :] = arr.reshape(P, 3, TILES).transpose(1, 2, 0)
    return out


# ---------------------------------------------------------------------------
# public entry point
# ---------------------------------------------------------------------------

def kernel(ctc, aif, time, seg, eta_nn, lambda_reg):
    from concourse.bass_utils import run_bass_kernel_spmd

    ctc = np.asarray(ctc)
    aif = np.asarray(aif)
    time = np.asarray(time)
    eta_nn = np.asarray(eta_nn)
    lambda_reg = np.asarray(lambda_reg)

    in_maps = _make_in_maps(ctc, aif, time, eta_nn, lambda_reg)
    nc = _build_nc()
    res = run_bass_kernel_spmd(nc, in_maps, list(range(N_CORES)))

    out = np.zeros((1, 3, H, W), np.float32)
    for m in range(N_CORES):
        rows = slice(m * ROWS_PER_CORE, (m + 1) * ROWS_PER_CORE)
        arr = res.results[m]["out"]                  # [128, 48]
        out[0, :, rows, :] = arr.reshape(P, 3, TILES).transpose(1, 2, 0)
    return out


# revision 6
# speedup vs baseline: 3.5095x; 1.6377x over previous
"""Trainium2 Bass kernel for the DeepFermi deconvolution GD problem.

Node-collapsed formulation: the per-pixel fermi sigmoid s1(tsh) is smooth on
the >=1 time-unit scale (k <= ~1), so the 512-point oversampled time grid is
replaced by Nc=16 interpolation nodes tshc; the piecewise-linear interp
matrix L is folded host-side into the convolution matrices:

    ML = M2 @ L, MVL = M2V @ L
    G  = ML^T ML,  Gv^T = ML^T MVL          (Nc x Nc Gram matrices)
    nw  = -(2/C_dc) ctc_dc @ ML             (per-pixel constants)
    nwv = -(2/C_dc) ctc_dc @ MVL

Per GD iteration, per pixel (s1c/sdc on the Nc nodes):
    w  = a2c*(G @ s1c)  + nw                (a2c = 2A/C_dc)
    wv = a2c*(Gv @ s1c) + nwv
    gA = w.s1c ; U = w.sdc ; V = wv.sdc
    gk = A(t0 U - V) ; gt0 = A k U          (+ prior & positivity terms)

Mapping: H sharded over 8 cores (16 rows each); tile = one H row = 128
pixels on partitions.  All 16 tiles are batched per iteration: tiles are
stacked along PE partitions for the transposed sigmoid (2 halves of 8
tiles x 16 nodes = 128), block-diagonal Gram matmuls produce y|yv for 8
tiles at once, and all DVE elementwise/reduce work runs at [128, 256-512]
free size.  eta columns are ordered A|t0|k so the k/t0 gradient + update +
kn-transpose (the only serial inter-iteration dependency) runs first;
the A-update and gA dot products happen in the shadow of the next
iteration's front end.
"""

import numpy as np

OSAMP = 8
MAX_ITER = 10
NEG_SHIFT = 2 * OSAMP
OTP = 5
C_SHARP = 500.0
LR = 0.1
T = 64
TOS = OSAMP * T  # 512
H = 128
W = 128
N_CORES = 8
ROWS_PER_CORE = H // N_CORES  # 16
TILES = ROWS_PER_CORE  # 16 tiles of 128 pixels per core
P = 128
NC = 16        # interpolation nodes
HGRP = 8       # tiles per half (stacked on PE partitions: 8*16 = 128)


# ---------------------------------------------------------------------------
# host-side math (iteration independent)
# ---------------------------------------------------------------------------

def _resize_mat(in_size, out_size):
    scale = out_size / in_size
    sample_f = (np.arange(out_size) + 0.5) / scale - 0.5
    x = np.abs(sample_f[None, :] - np.arange(in_size)[:, None])
    w = np.maximum(0.0, 1.0 - x)
    tot = w.sum(0, keepdims=True)
    w = np.where(np.abs(tot) > 1e-4, w / tot, 0.0)
    return w  # float64


def _sigmoid(x):
    with np.errstate(over="ignore"):
        return 1.0 / (1.0 + np.exp(-x))


def _interp_mat(tsh, tshc):
    Nc = len(tshc)
    Lm = np.zeros((len(tsh), Nc))
    j = np.searchsorted(tshc, tsh)
    for v, (x, jj) in enumerate(zip(tsh, j)):
        if jj <= 0:
            Lm[v, 0] = 1.0
        elif jj >= Nc:
            Lm[v, Nc - 1] = 1.0
        else:
            x0, x1 = tshc[jj - 1], tshc[jj]
            a = (x - x0) / (x1 - x0)
            Lm[v, jj - 1] = 1.0 - a
            Lm[v, jj] = a
    return Lm


def _preprocess(ctc, aif, time, eta_nn, lambda_reg):
    f64 = np.float64
    R = _resize_mat(T, TOS)
    aif0 = (aif.astype(f64) - aif.astype(f64)[..., :OTP].mean(-1, keepdims=True))
    ctc0 = (ctc.astype(f64) - ctc.astype(f64)[..., :OTP].mean(-1, keepdims=True))
    aif_os = (aif0 @ R)[0, 0, 0]                    # [512]
    t_os = time.astype(f64) @ R                     # [512]
    ctc_dc = (ctc0 @ R[:, ::OSAMP])[0]              # [H,W,64]
    C_dc = float((ctc_dc.astype(np.float32) ** 2).sum(dtype=np.float64))
    tsh = t_os - t_os[NEG_SHIFT]
    s2 = _sigmoid((C_SHARP * tsh).astype(np.float32).astype(f64))
    idx = NEG_SHIFT + 8 * np.arange(T)[:, None] - np.arange(TOS)[None, :]
    valid = (idx >= 0) & (idx <= TOS - 1)
    M = np.where(valid, aif_os[np.clip(idx, 0, TOS - 1)], 0.0) / OSAMP  # [64,512]
    M2 = M * s2[None, :]
    M2V = M2 * tsh[None, :]

    import ml_dtypes
    nidx = np.round(np.linspace(0, TOS - 1, NC)).astype(int)
    # bf16-representable node values so device arg matches host L exactly
    tshc = tsh[nidx].astype(ml_dtypes.bfloat16).astype(f64)
    Lm = _interp_mat(tsh, tshc)
    ML = M2 @ Lm                                    # [64, NC]
    MVL = M2V @ Lm
    G = ML.T @ ML                                   # [NC, NC] symmetric
    GvT = ML.T @ MVL                                # y_v = s1c @ GvT
    nw = -(2.0 / C_dc) * np.einsum('hwj,jc->hwc', ctc_dc, ML)    # [H,W,NC]
    nwv = -(2.0 / C_dc) * np.einsum('hwj,jc->hwc', ctc_dc, MVL)

    C_nn = (eta_nn.astype(f64) ** 2).sum(axis=(0, 2, 3))  # [3]
    sp_lam = np.logaddexp(0.0, float(lambda_reg.reshape(-1)[0]))
    creg = 2.0 * sp_lam / C_nn                      # [3]
    return tshc, G, GvT, nw, nwv, C_dc, creg


# ---------------------------------------------------------------------------
# bass module
# ---------------------------------------------------------------------------

_NC_CACHE = {}


def _build_nc():
    if "nc" in _NC_CACHE:
        return _NC_CACHE["nc"]

    import concourse.mybir as mybir
    import concourse.tile as tile
    from concourse import bacc

    dt = mybir.dt.float32
    bf = mybir.dt.bfloat16
    Alu = mybir.AluOpType
    Act = mybir.ActivationFunctionType
    X = mybir.AxisListType.X

    nc = bacc.Bacc("TRN2", target_bir_lowering=False, debug=False)

    # shared constants
    d_argwbd = nc.declare_dram_parameter("argwbd", [2 * TILES, TILES * NC],
                                         bf, isOutput=False)
    d_ggvd = nc.declare_dram_parameter("ggvd", [HGRP * NC, HGRP * 2 * NC], bf,
                                       isOutput=False)
    d_ident = nc.declare_dram_parameter("ident", [P, P], bf, isOutput=False)
    # per-core data
    d_nwfull = nc.declare_dram_parameter("nwfull", [P, TILES * 2 * NC], bf,
                                         isOutput=False)
    d_eta0 = nc.declare_dram_parameter("eta0", [P, 3 * TILES], dt, isOutput=False)
    d_cpl48 = nc.declare_dram_parameter("cpl48", [P, 3 * TILES], dt, isOutput=False)
    d_s48 = nc.declare_dram_parameter("s48", [P, 3 * TILES], dt, isOutput=False)
    d_toc = nc.declare_dram_parameter("toc", [P, TILES], dt, isOutput=False)
    d_out = nc.declare_dram_parameter("out", [P, 3 * TILES], dt, isOutput=True)

    NT2 = 2 * NC * TILES  # 512: w|wv free size for all tiles
    NT1 = NC * TILES      # 256

    with tile.TileContext(nc) as tc:
        with (
            tc.tile_pool(name="const", bufs=1) as cpool,
            tc.tile_pool(name="state", bufs=2) as spool,
            tc.tile_pool(name="work", bufs=2) as wpool,
            tc.tile_pool(name="small", bufs=2) as mpool,
            tc.tile_pool(name="ps_a", bufs=2, space="PSUM") as ps_a,
            tc.tile_pool(name="ps_b", bufs=2, space="PSUM") as ps_b,
            tc.tile_pool(name="ps_y", bufs=2, space="PSUM") as ps_y,
            tc.tile_pool(name="ps_k", bufs=1, space="PSUM") as ps_k,
        ):
            # ---- load constants (order matters: first-needed first) ----
            ident = cpool.tile([P, P], bf, tag="ident")
            nc.gpsimd.dma_start(ident[:], d_ident[:])
            eta_in = cpool.tile([P, 3 * TILES], dt, tag="eta_in")
            nc.gpsimd.dma_start(eta_in[:], d_eta0[:])
            toc = cpool.tile([P, TILES], dt, tag="toc")
            nc.gpsimd.dma_start(toc[:], d_toc[:])
            argwbd = cpool.tile([2 * TILES, TILES * NC], bf, tag="argwbd")
            nc.gpsimd.dma_start(argwbd[:], d_argwbd[:])
            ggvd = cpool.tile([HGRP * NC, HGRP * 2 * NC], bf, tag="ggvd")
            nc.gpsimd.dma_start(ggvd[:], d_ggvd[:])
            s48 = cpool.tile([P, 3 * TILES], dt, tag="s48")
            nc.gpsimd.dma_start(s48[:], d_s48[:])
            cpl48 = cpool.tile([P, 3 * TILES], dt, tag="cpl48")
            nc.gpsimd.dma_start(cpl48[:], d_cpl48[:])
            nwfull = cpool.tile([P, TILES * 2 * NC], bf, tag="nwfull")
            nc.gpsimd.dma_start(nwfull[:], d_nwfull[:])

            eta48 = eta_in

            def make_derived(e48):
                # eta columns: A | t0 | k
                eK = e48[:, 2 * TILES:3 * TILES]
                eT = e48[:, TILES:2 * TILES]
                kn = spool.tile([P, 2 * TILES], bf, tag="kn")
                nc.vector.tensor_tensor(kn[:, 0:2 * TILES:2], eK, eT, Alu.mult)
                nc.vector.tensor_scalar_mul(kn[:, 1:2 * TILES:2], eK, -1.0)
                knt_ps = ps_k.tile([2 * TILES, P], bf, tag="kntp")
                nc.tensor.transpose(knt_ps[:], kn[:], ident[:])
                knT = spool.tile([2 * TILES, P], bf, tag="knT")
                nc.scalar.copy(knT[:], knt_ps[:])
                return knT

            def make_a2c(e48):
                a2c = spool.tile([P, TILES], dt, tag="a2c")
                nc.vector.tensor_tensor(a2c[:], e48[:, 0:TILES], toc[:],
                                        Alu.mult)
                return a2c

            knT = make_derived(eta48)
            a2c = make_a2c(eta48)

            for it in range(MAX_ITER):
                # ---- shadow: X48 = eta*s48 + m48 + cpl48 (GpSimd) ----
                m48 = mpool.tile([P, 3 * TILES], dt, tag="m48")
                nc.gpsimd.tensor_scalar(m48[:], eta48[:], 0.0, -2.0 * LR,
                                        Alu.min, Alu.mult)
                up48 = mpool.tile([P, 3 * TILES], dt, tag="up48")
                nc.gpsimd.tensor_tensor(up48[:], eta48[:], s48[:], Alu.mult)
                xb = mpool.tile([P, 3 * TILES], dt, tag="xb")
                nc.gpsimd.tensor_tensor(xb[:], up48[:], m48[:], Alu.add)
                X48 = mpool.tile([P, 3 * TILES], dt, tag="X48")
                nc.gpsimd.tensor_tensor(X48[:], xb[:], cpl48[:], Alu.add)

                # ---- pixel-major arg + sigmoid + sdc (off critical path) ----
                arg2 = ps_b.tile([P, NT1], dt, tag="arg2")
                nc.tensor.matmul(arg2[:], knT[:], argwbd[:], start=True,
                                 stop=True)
                s1c = wpool.tile([P, NT1], bf, tag="s1c")
                nc.scalar.activation(s1c[:], arg2[:], Act.Sigmoid)
                sdc = wpool.tile([P, NT1], bf, tag="sdc")
                sdacc = wpool.tile([P, 1], dt, tag="sdacc")
                nc.vector.affine_mul_reduce(sdc[:], sdacc[:], s1c[:], s1c[:],
                                            -1.0, 1.0)

                # ---- transposed sigmoid + Gram matmuls, two halves ----
                yps = ps_y.tile([P, NT2], dt, tag="yps")
                for h in range(2):
                    argT = ps_a.tile([P, P], dt, tag="argT")
                    nc.tensor.matmul(argT[:],
                                     argwbd[:, h * P:(h + 1) * P], knT[:],
                                     start=True, stop=True)
                    s1cT = wpool.tile([P, P], bf, tag="s1cT")
                    nc.scalar.activation(s1cT[:], argT[:], Act.Sigmoid)
                    nc.tensor.matmul(yps[:, h * NT1:(h + 1) * NT1],
                                     s1cT[:], ggvd[:], start=True, stop=True)

                # ---- w|wv = a2c*y + nw (PSUM read, broadcast a2c) ----
                a2cB = a2c[:].unsqueeze(2).broadcast_to([P, TILES, 2 * NC])
                w1 = wpool.tile([P, NT2], bf, tag="w1")
                w1v = w1[:].rearrange("p (t n) -> p t n", t=TILES)
                ypsv = yps[:].rearrange("p (t n) -> p t n", t=TILES)
                nc.vector.tensor_tensor(w1v, ypsv, a2cB, Alu.mult)
                wg = wpool.tile([P, NT2], bf, tag="wg")
                nc.vector.tensor_tensor(wg[:], w1[:], nwfull[:], Alu.add)

                # ---- products + segmented reduces ----
                w4 = wg[:].rearrange("p (t u c) -> p t u c", t=TILES, u=2,
                                     c=NC)
                sdc4 = sdc[:].rearrange("p (t c) -> p t c", t=TILES)\
                    .unsqueeze(2).broadcast_to([P, TILES, 2, NC])
                pA = wpool.tile([P, NT2], bf, tag="pA")
                pA4 = pA[:].rearrange("p (t u c) -> p t u c", t=TILES, u=2,
                                      c=NC)
                nc.vector.tensor_tensor(pA4, w4, sdc4, Alu.mult)
                UVm = mpool.tile([P, 2 * TILES], dt, tag="UVm")
                nc.vector.reduce_sum(
                    UVm[:].rearrange("p (t u) -> p t u", t=TILES), pA4, axis=X)

                # ---- critical path: k/t0 gradient, update, kn transpose ----
                eA = eta48[:, 0:TILES]
                eT = eta48[:, TILES:2 * TILES]
                eK = eta48[:, 2 * TILES:3 * TILES]
                p12 = mpool.tile([P, 2 * TILES], dt, tag="p12")
                eAB = eA.unsqueeze(2).broadcast_to([P, TILES, 2])
                nc.vector.tensor_tensor(
                    p12[:].rearrange("p (t u) -> p t u", t=TILES),
                    UVm[:].rearrange("p (t u) -> p t u", t=TILES),
                    eAB, Alu.mult)
                p1 = p12[:, 0:2 * TILES:2]
                p2 = p12[:, 1:2 * TILES:2]
                gkt = mpool.tile([P, 2 * TILES], dt, tag="gkt")
                # cols 0:16 = gt0 = k*p1 ; cols 16:32 = gk = t0*p1 - p2
                nc.vector.tensor_tensor(gkt[:, 0:TILES], p1, eK, Alu.mult)
                wk16 = mpool.tile([P, TILES], dt, tag="wk16")
                nc.vector.tensor_tensor(wk16[:], p1, eT, Alu.mult)
                nc.vector.tensor_tensor(gkt[:, TILES:2 * TILES], wk16[:], p2,
                                        Alu.subtract)
                eta48n = spool.tile([P, 3 * TILES], dt, tag="eta48")
                nc.vector.affine_then_add(eta48n[:, TILES:3 * TILES], gkt[:],
                                          X48[:, TILES:3 * TILES], -LR, 0.0)
                if it < MAX_ITER - 1:
                    knT = make_derived(eta48n)

                # ---- shadow: gA dot, A update, a2c for next iter ----
                pB = wpool.tile([P, NT1], bf, tag="pB")
                pB3 = pB[:].rearrange("p (t c) -> p t c", t=TILES)
                s1c3 = s1c[:].rearrange("p (t c) -> p t c", t=TILES)
                nc.vector.tensor_tensor(pB3, w4[:, :, 0, :], s1c3, Alu.mult)
                gA = mpool.tile([P, TILES], dt, tag="gA")
                nc.vector.reduce_sum(gA[:], pB3, axis=X)
                nc.vector.affine_then_add(eta48n[:, 0:TILES], gA[:],
                                          X48[:, 0:TILES], -LR, 0.0)
                eta48 = eta48n
                if it < MAX_ITER - 1:
                    a2c = make_a2c(eta48)

            nc.gpsimd.dma_start(d_out[:], eta48[:])

    nc.finalize()
    _NC_CACHE["nc"] = nc
    return nc


# ---------------------------------------------------------------------------
# input staging (eta column order: A | t0 | k)
# ---------------------------------------------------------------------------

def _make_in_maps(ctc, aif, time, eta_nn, lambda_reg):
    f32 = np.float32
    import ml_dtypes
    bf16 = ml_dtypes.bfloat16

    tshc, G, GvT, nw, nwv, C_dc, creg = _preprocess(
        ctc, aif, time, eta_nn, lambda_reg)

    toc_v = 2.0 / C_dc
    sA, sK, sT0 = (1.0 - LR * creg).astype(np.float64)

    # argwbd[k, 16*t + c]: rows 2t -> 1.0, 2t+1 -> tshc[c]
    argwbd = np.zeros((2 * TILES, TILES * NC), bf16)
    tshcf = tshc.astype(f32)
    for t_ in range(TILES):
        argwbd[2 * t_, t_ * NC:(t_ + 1) * NC] = 1.0
        argwbd[2 * t_ + 1, t_ * NC:(t_ + 1) * NC] = tshcf
    # ggvd block-diag (8 tiles per half): [16,32] blocks of [G | Gv^T]
    ggvd = np.zeros((HGRP * NC, HGRP * 2 * NC), bf16)
    blk = np.concatenate([G, GvT], axis=1)          # [NC, 2*NC]
    for tau in range(HGRP):
        ggvd[tau * NC:(tau + 1) * NC, tau * 2 * NC:(tau + 1) * 2 * NC] = blk
    ident = np.eye(P, dtype=bf16)

    toc = np.full((P, TILES), toc_v, f32)
    s48 = np.zeros((P, 3 * TILES), f32)
    s48[:, 0:TILES] = sA          # A
    s48[:, TILES:2 * TILES] = sT0  # t0
    s48[:, 2 * TILES:] = sK        # k

    in_maps = []
    for m in range(N_CORES):
        rows = slice(m * ROWS_PER_CORE, (m + 1) * ROWS_PER_CORE)
        nwc = np.stack([nw[rows], nwv[rows]], axis=2)  # [16,128,2,NC]
        nwfull = np.ascontiguousarray(
            nwc.transpose(1, 0, 2, 3).reshape(P, TILES * 2 * NC)).astype(bf16)
        pr = eta_nn[0, :, rows, :].astype(np.float64)   # [3(A,k,t0), 16, 128]
        pr_atk = pr[[0, 2, 1]]                          # A | t0 | k
        eta0 = np.ascontiguousarray(
            pr_atk.transpose(2, 0, 1).reshape(P, 3 * TILES)).astype(f32)
        creg_atk = creg[[0, 2, 1]]
        cpl48 = np.zeros((P, 3 * TILES), f32)
        for c in range(3):
            cpl48[:, c * TILES:(c + 1) * TILES] = (
                LR * creg_atk[c] * pr_atk[c]).T
        in_maps.append({
            "argwbd": argwbd, "ggvd": ggvd, "ident": ident,
            "nwfull": nwfull, "eta0": eta0, "cpl48": cpl48, "s48": s48,
            "toc": toc,
        })
    return in_maps


def _emulate(in_maps):
    """Numpy replay of the device pipeline from staged arrays (debug aid)."""
    import ml_dtypes
    bf16 = ml_dtypes.bfloat16
    f32 = np.float32

    def bfq(x):
        return np.asarray(x, dtype=f32).astype(bf16).astype(f32)

    outs = []
    for mp in in_maps:
        argwbd = mp["argwbd"].astype(f32)
        ggvd = mp["ggvd"].astype(f32)
        nwfull = mp["nwfull"].astype(f32)
        eta48 = mp["eta0"].astype(f32).copy()
        cpl48 = mp["cpl48"]
        s48 = mp["s48"]
        toc = mp["toc"]
        for it in range(MAX_ITER):
            eA = eta48[:, 0:TILES]
            eT = eta48[:, TILES:2 * TILES]
            eK = eta48[:, 2 * TILES:]
            kn = np.zeros((P, 2 * TILES), f32)
            kn[:, 0::2] = bfq(eK * eT)
            kn[:, 1::2] = bfq(-eK)
            knT = kn.T  # [32, 128]
            a2c = eA * toc
            X48 = eta48 * s48 + np.minimum(eta48, 0.0) * (-2.0 * LR) + cpl48
            arg2 = knT.T @ argwbd          # [128, 256]
            s1c = bfq(_sigmoid(arg2))
            sdc = bfq(s1c * (1.0 - s1c))
            yps = np.zeros((P, 2 * NC * TILES), f32)
            for h in range(2):
                argT = argwbd[:, h * P:(h + 1) * P].T @ knT   # [128, 128]
                s1cT = bfq(_sigmoid(argT))
                yps[:, h * 2 * NC * HGRP:(h + 1) * 2 * NC * HGRP] = \
                    s1cT.T @ ggvd
            w1 = bfq(yps.reshape(P, TILES, 2 * NC)
                     * a2c[:, :, None]).reshape(P, -1)
            wg = bfq(w1 + nwfull)
            w4 = wg.reshape(P, TILES, 2, NC)
            pA = bfq(w4 * sdc.reshape(P, TILES, 1, NC))
            UV = pA.sum(-1)                 # [128, 16, 2]
            p1 = eA * UV[:, :, 0]
            p2 = eA * UV[:, :, 1]
            gt0 = p1 * eK
            gk = p1 * eT - p2
            pB = bfq(w4[:, :, 0, :] * s1c.reshape(P, TILES, NC))
            gA = pB.sum(-1)
            G48 = np.concatenate([gA, gt0, gk], axis=1)
            eta48 = X48 - LR * G48
        outs.append(eta48)
    out = np.zeros((1, 3, H, W), f32)
    for m, arr in enumerate(outs):
        rows = slice(m * ROWS_PER_CORE, (m + 1) * ROWS_PER_CORE)
        a3 = arr.reshape(P, 3, TILES)      # A | t0 | k
        out[0, 0, rows, :] = a3[:, 0, :].T
        out[0, 1, rows, :] = a3[:, 2, :].T
        out[0, 2, rows, :] = a3[:, 1, :].T
    return out


# ---------------------------------------------------------------------------
# public entry point
# ---------------------------------------------------------------------------

def kernel(ctc, aif, time, seg, eta_nn, lambda_reg):
    from concourse.bass_utils import run_bass_kernel_spmd

    ctc = np.asarray(ctc)
    aif = np.asarray(aif)
    time = np.asarray(time)
    eta_nn = np.asarray(eta_nn)
    lambda_reg = np.asarray(lambda_reg)

    in_maps = _make_in_maps(ctc, aif, time, eta_nn, lambda_reg)
    nc = _build_nc()
    res = run_bass_kernel_spmd(nc, in_maps, list(range(N_CORES)))

    out = np.zeros((1, 3, H, W), np.float32)
    for m in range(N_CORES):
        rows = slice(m * ROWS_PER_CORE, (m + 1) * ROWS_PER_CORE)
        arr = res.results[m]["out"]                  # [128, 48] A|t0|k
        a3 = arr.reshape(P, 3, TILES)
        out[0, 0, rows, :] = a3[:, 0, :].T
        out[0, 1, rows, :] = a3[:, 2, :].T
        out[0, 2, rows, :] = a3[:, 1, :].T
    return out
